# revision 18
# baseline (speedup 1.0000x reference)
"""Trainium2 Bass kernel for a 2-layer GATv2 (nn_GAT_40372692582770).

Gather-free, PE-centric design (v2):
  - Global 64-dst strips LPT-assigned to 8 cores (balances both total load
    and the per-position sorted-count profile, so the shared SPMD block
    schedule sched[i] = max_c cnt_sorted[c][i] has ~2-4% padding).
  - Host ships, per layer, per-edge feature columns (halo exchange
    materialized host-side; the graph is static):
      xsd [128, S]       = x[src_e] / x[dst_e] columns           (fp16)
      xeP [128, NBLK*66] = x[src_e] rows + valid col (P-major)   (fp16)
      ohP [128, NBLK*64] = one-hot(dstloc) precomputed           (fp8, exact)
  - Device pipeline, 2-iteration software stages (per pair of strips):
      frontA(p): z = Wsd^T xsd (PE, 512-col groups) ->
                 L = lrelu(z) split Scalar(Prelu) / Vector(2-op relu trick)
      mid(q=p-2): e-dot blocks (lhsT=L_blk, rhs=A) INTERLEAVED with the
                 gt scatter chain of pair q-2 (hides e-dot LDWEIGHTS under
                 gt matmul execution); then w = exp(e-2) (Scalar);
                 woh = oh * w (Vector/GpSimd split)
      gt chain:  gt[j,128] += xe_blk^T @ woh_blk   (PE, per strip, PSUM)
      back2(r=p-5): sp[128,130] = sum_h gts_h^T @ R2_h (PE)
                 cols = [num_h0 | num_h1 | den_0 den_1]
  - Finalize (interleaved in batches): alpha-normalize, head-mean, +bias,
    gelu -> out_raw fp32 + out_act fp16, [128, NS*C] P-major in core-slot
    order (host unpermutes via the strip assignment).

One program serves both layers (weights are inputs); compiled once per
(block schedule, has_bias).
"""
import os
import sys
import time

sys.path.insert(0, "/opt/trn_rl_repo")

import numpy as np

import concourse.bass as bass
import concourse.mybir as mybir
import concourse.tile as tile
from concourse import bacc
from concourse.bass_utils import run_bass_kernel_spmd

class Cfg:
    N = 100000
    D = 64
    H = 2
    C = 64
    NCORES = 8
    W = 64             # dst nodes per strip
    ESHIFT = -2.0      # exp bias
    NFIN = 8           # finalize batches

    @property
    def GSTRIP(self):
        return (self.N + self.W - 1) // self.W   # global strips (1563)

    @property
    def NSTRIP(self):
        return (self.GSTRIP + self.NCORES - 1) // self.NCORES  # per core (196)

    @property
    def NPAIR(self):
        return self.NSTRIP // 2

    @property
    def HC(self):
        return self.H * self.C


CFG = Cfg()
FP16 = mybir.dt.float16
FP32 = mybir.dt.float32
FP8 = mybir.dt.float8e4
NP8 = mybir.dt.np(mybir.dt.float8e4)
AF = mybir.ActivationFunctionType
ALU = mybir.AluOpType
PM = mybir.MatmulPerfMode


# ------------------------------------------------------------- host prep
def _prep_edges(cfg, src, dst):
    """Global LPT strip->core assignment; route+sort edges; shared schedule.

    Returns (sched [NSTRIP], assign [NCORES, NSTRIP] of global strip ids
    (-1 = empty pad slot), slots: per-core (srcids [S], dloc [S]);
    pad slots src=-1 dloc=255).
    """
    GS = cfg.GSTRIP
    NSTRIP = cfg.NSTRIP
    shift = int(np.log2(cfg.W))
    strip = (dst >> shift).astype(np.int64)
    scnt = np.bincount(strip, minlength=GS).astype(np.int64)
    order = np.argsort(-scnt, kind="stable")
    loads = np.zeros(cfg.NCORES, np.int64)
    nst = np.zeros(cfg.NCORES, np.int64)
    assign = np.full((cfg.NCORES, NSTRIP), -1, np.int64)
    core_of_strip = np.zeros(GS, np.int64)
    BIG = np.int64(1) << 60
    for gs in order:
        cand = np.where(nst < NSTRIP, loads, BIG)
        c = int(np.argmin(cand))
        assign[c, nst[c]] = gs
        core_of_strip[gs] = c
        loads[c] += scnt[gs]
        nst[c] += 1
    # per-core slot counts (assign rows are in decreasing-count order)
    cnt_sorted = np.where(assign >= 0, scnt[np.clip(assign, 0, GS - 1)], 0)
    sched = np.maximum(1, (cnt_sorted.max(axis=0) + 127) // 128).astype(int)
    boff = np.concatenate([[0], np.cumsum(sched)])
    nslot = int(boff[-1]) * 128

    slot_of_strip = np.zeros(GS, np.int64)
    for c in range(cfg.NCORES):
        sel = assign[c] >= 0
        slot_of_strip[assign[c][sel]] = np.flatnonzero(sel)

    ecore = core_of_strip[strip]
    eslot = slot_of_strip[strip]
    dloc_all = (dst & (cfg.W - 1)).astype(np.int64)
    out = []
    for c in range(cfg.NCORES):
        sel = np.flatnonzero(ecore == c)
        o = np.argsort(eslot[sel], kind="stable")
        sel = sel[o]
        s = src[sel].astype(np.int64)
        d = dloc_all[sel]
        islot = eslot[sel]
        # position within each slot-run
        runstart = np.concatenate([[0], np.cumsum(
            np.bincount(islot, minlength=NSTRIP))])[:-1]
        pos = np.arange(len(sel)) - runstart[islot]
        dest = boff[islot] * 128 + pos
        sids = np.full(nslot, -1, np.int64)
        dloc = np.full(nslot, 255, np.int64)
        sids[dest] = s
        dloc[dest] = d
        out.append((sids, dloc))
    return sched, assign, out


def _prep_layer_weights(cfg, Wl, bl, Wr, br, att, bias):
    D, H, C = cfg.D, cfg.H, cfg.C
    HC = cfg.HC
    wsd = np.concatenate([Wl, Wr], axis=0)  # [128, HC]
    pbias = (bl + br).astype(np.float32)[:, None]
    A = np.zeros((HC, H), np.float64)
    for h in range(H):
        A[h * C:(h + 1) * C, h] = att[h]
    R2 = np.zeros((H, 66, 130), np.float64)
    for h in range(H):
        R2[h, :D, h * C:(h + 1) * C] = Wl[:, h * C:(h + 1) * C]
        R2[h, D, h * C:(h + 1) * C] = bl[h * C:(h + 1) * C]
        R2[h, D, HC + h] = 1.0          # denominator column
    biasF = np.tile(bias.astype(np.float32)[None, :], (128, 1))
    return {
        "wsd": wsd.astype(np.float16), "pbias": pbias,
        "A": A.astype(np.float16),
        "R2_0": R2[0].astype(np.float16), "R2_1": R2[1].astype(np.float16),
        "biasF": biasF,
    }


# --------------------------------------------------------- program build
def build_program(cfg, sched, has_bias, final_layer):
    D, H, C = cfg.D, cfg.H, cfg.C
    HC = cfg.HC
    W = cfg.W
    sched = [int(b) for b in sched]
    boff = [0]
    for b in sched:
        boff.append(boff[-1] + b)
    NBLK = boff[-1]
    S = NBLK * 128
    NS = cfg.NPAIR
    RROW = HC + 2
    BPMAX = max(sched[2 * p] + sched[2 * p + 1] for p in range(NS))
    MBA = max(sched)
    CSMAX = BPMAX * 128

    # finalize batches (front-loaded so the post-loop tail is small)
    fin_after = {}
    p0 = 0
    for frac in (16, 32, 48, 62, 74, 84, 90, 95, 98):
        pe_ = (NS * frac) // 98
        if pe_ > p0:
            fin_after[pe_ - 1] = (p0, pe_)
            p0 = pe_
    assert p0 == NS

    nc = bacc.Bacc("TRN2", target_bir_lowering=False, debug=False,
                   num_devices=cfg.NCORES)

    xsd = nc.declare_dram_parameter("xsd", [2 * D, S], FP16, isOutput=False)
    xeP = nc.declare_dram_parameter("xeP", [128, NBLK * 66], FP16, isOutput=False)
    ohD = nc.declare_dram_parameter("ohP", [128, NBLK * W], FP8, isOutput=False)
    wsd = nc.declare_dram_parameter("wsd", [2 * D, HC], FP16, isOutput=False)
    pbias = nc.declare_dram_parameter("pbias", [HC, 1], FP32, isOutput=False)
    Amat = nc.declare_dram_parameter("A", [HC, H], FP16, isOutput=False)
    R2_0 = nc.declare_dram_parameter("R2_0", [66, 130], FP16, isOutput=False)
    R2_1 = nc.declare_dram_parameter("R2_1", [66, 130], FP16, isOutput=False)
    biasF = nc.declare_dram_parameter("biasF", [128, C], FP32, isOutput=False)
    out_raw = nc.declare_dram_parameter("out_raw", [128, NS * C], FP32,
                                        isOutput=True)
    out_act = nc.declare_dram_parameter("out_act", [128, NS * C], FP16,
                                        isOutput=True)

    with tile.TileContext(nc) as tc:
        with (
            tc.tile_pool(name="const", bufs=1) as cpool,
            tc.tile_pool(name="stash", bufs=1) as stpool,
            tc.tile_pool(name="fin", bufs=1) as fpool,
        ):
            wsd_t = cpool.tile([2 * D, HC], FP16)
            nc.sync.dma_start(out=wsd_t[:], in_=wsd[:, :])
            pbias_t = cpool.tile([HC, 1], FP32)
            nc.sync.dma_start(out=pbias_t[:], in_=pbias[:, :])
            A_t = cpool.tile([HC, H], FP16)
            nc.sync.dma_start(out=A_t[:], in_=Amat[:, :])
            r2_t = [cpool.tile([66, 130], FP16, tag=f"r2{h}", name=f"r2{h}")
                    for h in range(H)]
            nc.sync.dma_start(out=r2_t[0][:], in_=R2_0[:, :])
            nc.sync.dma_start(out=r2_t[1][:], in_=R2_1[:, :])
            ebias_t = cpool.tile([128, 1], FP32)
            nc.vector.memset(ebias_t[:], cfg.ESHIFT)
            bias_t = cpool.tile([128, C], FP32)
            nc.sync.dma_start(out=bias_t[:], in_=biasF[:, :])
            # warm the ACT tables during initial DMA (Prelu/Exp/Sigmoid
            # loads otherwise stall the first pipeline iterations)
            warm_t = cpool.tile([128, 4], FP32)
            nc.vector.memset(warm_t[:, 0:1], 0.0)
            nc.scalar.activation(out=warm_t[:, 1:2], in_=warm_t[:, 0:1],
                                 func=AF.Prelu, alpha=0.2)
            nc.scalar.activation(out=warm_t[:, 2:3], in_=warm_t[:, 0:1],
                                 func=AF.Exp)
            nc.scalar.copy(warm_t[:, 3:4], warm_t[:, 0:1])
            if not final_layer:
                nc.scalar.activation(out=warm_t[:, 3:4], in_=warm_t[:, 0:1],
                                     func=AF.Sigmoid)
                nc.scalar.square(warm_t[:, 3:4], warm_t[:, 0:1])

            stash = stpool.tile([128, NS * RROW], FP32)
            sv = stash[:].rearrange("p (s w) -> p s w", w=RROW)

            NSBMAX = max(p1 - p0 for p0, p1 in fin_after.values())

            with (
                tc.tile_pool(name="eg", bufs=7) as egpool,
                tc.tile_pool(name="ez", bufs=7) as ezpool,
                tc.tile_pool(name="esm", bufs=6) as smpool,
                tc.tile_pool(name="fb", bufs=2) as fbpool,
                tc.tile_pool(name="zps", bufs=2, space="PSUM") as zpspool,
                tc.tile_pool(name="eps", bufs=1, space="PSUM") as epspool,
                tc.tile_pool(name="gps", bufs=2, space="PSUM") as gpspool,
                tc.tile_pool(name="sps", bufs=1, space="PSUM") as spspool,
            ):
                def frontA(pr):
                    bA = sched[2 * pr]
                    bB = sched[2 * pr + 1]
                    bp = bA + bB
                    b0 = boff[2 * pr]
                    c0 = b0 * 128
                    CS = bp * 128
                    xsd_t = egpool.tile([2 * D, CSMAX], FP16, tag="xsd")
                    nc.sync.dma_start(out=xsd_t[:, :CS],
                                      in_=xsd[:, c0:c0 + CS])
                    xe_t = egpool.tile([128, BPMAX * 66], FP16, tag="xe")
                    nc.sync.dma_start(out=xe_t[:, :bp * 66],
                                      in_=xeP[:, b0 * 66:(b0 + bp) * 66])
                    xev = xe_t[:, :bp * 66].rearrange("p (b w) -> p b w", w=66)
                    oh_t = egpool.tile([128, BPMAX * W], FP8, tag="oh")
                    nc.sync.dma_start(out=oh_t[:, :bp * W],
                                      in_=ohD[:, b0 * W:(b0 + bp) * W])
                    ohv = oh_t[:, :bp * W].rearrange("p (b s) -> p b s", s=W)

                    # zT feature-major; matmuls in <=512-col chunks (one
                    # PSUM bank each), lrelu over 1024-col double-bank
                    # tiles to halve Scalar per-op init overhead
                    L = ezpool.tile([128, CSMAX], FP16, tag="L")
                    ngrp = (CS + 1023) // 1024
                    for g in range(ngrp):
                        g0 = g * 1024
                        gw = min(1024, CS - g0)
                        zp = zpspool.tile([128, 1024], FP32, tag="zp")
                        for h0 in range(0, gw, 512):
                            hw_ = min(512, gw - h0)
                            nc.tensor.matmul(zp[:, h0:h0 + hw_],
                                             lhsT=wsd_t[:],
                                             rhs=xsd_t[:, g0 + h0:g0 + h0 + hw_],
                                             start=True, stop=True)
                        nc.scalar.activation(out=L[:, g0:g0 + gw],
                                             in_=zp[:, :gw], func=AF.Prelu,
                                             bias=pbias_t[:], alpha=0.2)
                    return (pr, bA, bB, xev, ohv, L, bp)

                def mid(stA, stW):
                    # e-dot blocks of pair q interleaved with gt chain of
                    # pair q-2; then exp(q), woh(q); gts-copy(q-2).
                    gts = None
                    ed_mms = []
                    ep = None
                    if stA is not None:
                        (pr, bA, bB, xev, ohv, L, bp) = stA
                        ep = epspool.tile([128, 2 * BPMAX], FP32, tag="ep")
                        for b in range(bp):
                            ed_mms.append((L, b))
                    gt_mms = []
                    gt_ps = {}
                    if stW is not None:
                        (prw, bAw, bBw, xevw, wohAvw, wohBvw, bVw) = stW

                        def whv(blk):
                            if blk < bVw:
                                return wohAvw[:, blk, :, :]
                            return wohBvw[:, blk - bVw, :, :]

                        gts = smpool.tile([66, 256], FP16, tag="gts")
                        for half, nb in ((0, bAw), (1, bBw)):
                            gt = gpspool.tile([66, 128], FP32, tag="gt")
                            gt_ps[half] = gt
                            base = 0 if half == 0 else bAw
                            for b in range(nb):
                                gt_mms.append((gt, base + b, b == 0,
                                               b == nb - 1))
                    # interleave on the PE queue: gt first (its matmul hides
                    # the following e-dot LDWEIGHTS)
                    ng, ne = len(gt_mms), len(ed_mms)
                    for i in range(max(ng, ne)):
                        if i < ng:
                            gt, blk, st, sp_ = gt_mms[i]
                            nc.tensor.matmul(
                                gt[:], lhsT=xevw[:, blk, :],
                                rhs=whv(blk), start=st, stop=sp_)
                        if i < ne:
                            L, b = ed_mms[i]
                            nc.tensor.matmul(ep[:, 2 * b:2 * b + 2],
                                             lhsT=L[:, b * 128:(b + 1) * 128],
                                             rhs=A_t[:], start=True, stop=True)
                    stW2 = None
                    if stA is not None:
                        (pr, bA, bB, xev, ohv, L, bp) = stA
                        w_t = smpool.tile([128, 2 * BPMAX], FP16, tag="w")
                        wv = w_t[:].rearrange("p (b k) -> p b k", k=2)
                        nc.scalar.activation(out=w_t[:, :2 * bp],
                                             in_=ep[:, :2 * bp], func=AF.Exp,
                                             bias=ebias_t[:])
                        # woh [128, b, 2, 64]  (V/G split by blocks)
                        bV = max(1, min(bp, int(round(bp * 0.55))))
                        bG = bp - bV
                        wohA = ezpool.tile([128, BPMAX * 2 * W], FP16,
                                           tag="wohA")
                        wohAv = wohA[:].rearrange("p (b h s) -> p b h s",
                                                  h=2, s=W)
                        wohB = ezpool.tile([128, BPMAX * 2 * W], FP16,
                                           tag="wohB")
                        wohBv = wohB[:].rearrange("p (b h s) -> p b h s",
                                                  h=2, s=W)
                        nc.vector.tensor_tensor(
                            out=wohAv[:, :bV, :, :],
                            in0=ohv[:, :bV, :].unsqueeze(2).to_broadcast(
                                [128, bV, 2, W]),
                            in1=wv[:, :bV, :].unsqueeze(3).to_broadcast(
                                [128, bV, 2, W]),
                            op=ALU.mult)
                        if bG > 0:
                            nc.gpsimd.tensor_tensor(
                                out=wohBv[:, :bG, :, :],
                                in0=ohv[:, bV:bp, :].unsqueeze(2).to_broadcast(
                                    [128, bG, 2, W]),
                                in1=wv[:, bV:bp, :].unsqueeze(3).to_broadcast(
                                    [128, bG, 2, W]),
                                op=ALU.mult)
                        stW2 = (pr, bA, bB, xev, wohAv, wohBv, bV)
                    # gts copy of pair q-2
                    if stW is not None:
                        gtsv = gts[:].rearrange("p (h s w) -> p h s w",
                                                h=2, s=2, w=W)
                        for half in (0, 1):
                            gtv = gt_ps[half][:].rearrange(
                                "p (h w) -> p h w", h=2, w=W)
                            dst_sl = gtsv[:, :, half, :]
                            if half == 0:
                                nc.vector.tensor_copy(dst_sl, gtv[:, :, :])
                            else:
                                nc.scalar.copy(dst_sl, gtv[:, :, :])
                        return (prw, gts), stW2
                    return None, stW2

                def back2(st2):
                    pr, gts = st2
                    sp = spspool.tile([128, RROW], FP32, tag="sp")
                    nc.tensor.matmul(sp[:], lhsT=gts[:, 0:128],
                                     rhs=r2_t[0][:], start=True, stop=False)
                    nc.tensor.matmul(sp[:], lhsT=gts[:, 128:256],
                                     rhs=r2_t[1][:], start=False, stop=True)
                    dst_sl = stash[:, pr * RROW:(pr + 1) * RROW]
                    nc.vector.tensor_copy(dst_sl, sp[:])

                def finalize(p0, p1):
                    NSb = p1 - p0
                    sl = slice(p0, p1)
                    rec = fbpool.tile([128, NSBMAX * 2], FP32, tag="rec")
                    recv = rec[:].rearrange("p (s k) -> p s k", k=2)
                    tmean = fbpool.tile([128, NSBMAX * C], FP32, tag="tm")
                    tmv = tmean[:].rearrange("p (s c) -> p s c", c=C)
                    tm = tmv[:, :NSb, :]
                    cub = fbpool.tile([128, NSBMAX * C], FP32, tag="cub")
                    cv = cub[:].rearrange("p (s c) -> p s c", c=C)[:, :NSb, :]
                    outg = fbpool.tile([128, NSBMAX * C], FP16, tag="outg")
                    ogv = outg[:].rearrange("p (s c) -> p s c", c=C)[:, :NSb, :]
                    nc.vector.reciprocal(out=recv[:, :NSb, :],
                                         in_=sv[:, sl, HC:HC + 2])
                    nc.vector.tensor_tensor(
                        out=tm[:, :, :], in0=sv[:, sl, 0:C],
                        in1=recv[:, :NSb, 0:1].to_broadcast([128, NSb, C]),
                        op=ALU.mult)
                    nc.gpsimd.tensor_tensor(
                        out=cv[:, :, :], in0=sv[:, sl, C:2 * C],
                        in1=recv[:, :NSb, 1:2].to_broadcast([128, NSb, C]),
                        op=ALU.mult)
                    nc.vector.tensor_tensor(out=tm[:, :, :],
                                            in0=tm[:, :, :],
                                            in1=cv[:, :, :], op=ALU.add)
                    # tm = 0.5*tm + bias
                    nc.vector.scalar_tensor_tensor(
                        out=tm[:, :, :], in0=tm[:, :, :], scalar=0.5,
                        in1=bias_t[:].unsqueeze(1).to_broadcast(
                            [128, NSb, C]),
                        op0=ALU.mult, op1=ALU.add)
                    if final_layer:
                        nc.scalar.dma_start(
                            out=out_raw[:, p0 * C:p1 * C],
                            in_=tmean[:, :NSb * C])
                        return
                    # gelu_tanh(x) = x*sigmoid(2*sqrt(2/pi)*(x+0.044715x^3))
                    nc.scalar.square(cv[:, :, :], tm[:, :, :])
                    nc.gpsimd.tensor_tensor(out=cv[:, :, :],
                                            in0=cv[:, :, :],
                                            in1=tm[:, :, :], op=ALU.mult)
                    nc.vector.scalar_tensor_tensor(
                        out=cv[:, :, :], in0=cv[:, :, :],
                        scalar=0.044715, in1=tm[:, :, :],
                        op0=ALU.mult, op1=ALU.add)
                    nc.scalar.activation(out=cv[:, :, :],
                                         in_=cv[:, :, :],
                                         func=AF.Sigmoid,
                                         scale=1.5957691216057308)
                    nc.vector.tensor_tensor(out=ogv[:, :, :],
                                            in0=cv[:, :, :],
                                            in1=tm[:, :, :], op=ALU.mult)
                    nc.scalar.dma_start(
                        out=out_act[:, p0 * C:p1 * C],
                        in_=outg[:, :NSb * C])

                # pipeline: frontA(i) | mid: edot(i-2)+gt(i-4) | back2(i-5)
                stA = {}
                stW = {}
                gts_q = {}
                for i in range(NS + 5):
                    if i < NS:
                        stA[i] = frontA(i)
                    q = i - 2
                    if 0 <= q < NS + 2:
                        a = stA.pop(q, None)
                        wst = stW.pop(q - 2, None)
                        g, w2 = mid(a, wst)
                        if w2 is not None:
                            stW[q] = w2
                        if g is not None:
                            gts_q[g[0]] = g
                    r = i - 4
                    if 0 <= r < NS and r in gts_q:
                        back2(gts_q.pop(r))
                        if r in fin_after:
                            finalize(*fin_after[r])
                # drain any stragglers (shouldn't happen, but be safe)
                for r in sorted(gts_q):
                    back2(gts_q.pop(r))
                    if r in fin_after:
                        finalize(*fin_after[r])

    nc.compile()
    return nc


# gt-half views: mid() passes (wohAv, wohBv, bV) — the gt chain maps
# logical block b to wohAv[b] if b < bV else wohBv[b - bV]. Half A spans
# blocks [0, bA), half B [bA, bp). Resolve in a helper at issue time.
def _woh_view(wohAv, wohBv, bV, blk):
    if blk < bV:
        return wohAv[:, blk, :, :]
    return wohBv[:, blk - bV, :, :]


# ----------------------------------------------------- persistent runner
class Runner:
    """Jit-compiled SPMD callable with reusable device inputs (no donation)."""

    def __init__(self, nc, n_cores):
        import jax
        import concourse.mybir as mb
        from concourse import bass2jax
        from jax.experimental.shard_map import shard_map
        from jax.sharding import Mesh, PartitionSpec
        bass2jax.install_neuronx_cc_hook()
        self.nc = nc
        self.n_cores = n_cores
        in_names, out_names, out_avals, zero_outs = [], [], [], []
        for alloc in nc.m.functions[0].allocations:
            if not isinstance(alloc, mb.MemoryLocationSet):
                continue
            name = alloc.memorylocations[0].name
            if alloc.kind == "ExternalInput":
                in_names.append(name)
            elif alloc.kind == "ExternalOutput":
                out_names.append(name)
                shape = tuple(alloc.tensor_shape)
                dtype = mb.dt.np(alloc.dtype)
                out_avals.append(jax.core.ShapedArray(shape, dtype))
                zero_outs.append(np.zeros(shape, dtype))
        pt = nc.partition_id_tensor
        self.pname = pt.name if pt else None
        if self.pname in in_names:
            in_names.remove(self.pname)
        self.in_names = list(in_names)
        self.out_names = list(out_names)
        self.out_avals = out_avals
        self.zero_outs = zero_outs
        all_in = list(in_names) + list(out_names)
        if self.pname:
            all_in.append(self.pname)

        def _body(*args):
            operands = list(args)
            if self.pname:
                operands.append(bass2jax.partition_id_tensor())
            outs = bass2jax._bass_exec_p.bind(
                *operands,
                out_avals=tuple(out_avals),
                in_names=tuple(all_in),
                out_names=tuple(out_names),
                lowering_input_output_aliases=(),
                sim_require_finite=True,
                sim_require_nnan=True,
                nc=nc,
            )
            return tuple(outs)

        devices = jax.devices()[:n_cores]
        self.mesh = Mesh(np.asarray(devices), ("core",))
        np_in = (PartitionSpec("core"),) * (len(in_names) + len(out_names))
        np_out = (PartitionSpec("core"),) * len(out_names)
        self.fn = jax.jit(shard_map(_body, mesh=self.mesh, in_specs=np_in,
                                    out_specs=np_out, check_rep=False),
                          keep_unused=True)

    def put(self, in_maps):
        """Concat per-core inputs and move to device. Returns arg list."""
        import jax
        from jax.sharding import NamedSharding, PartitionSpec
        sh = NamedSharding(self.mesh, PartitionSpec("core"))
        args = []
        for name in self.in_names:
            cat = np.concatenate([np.asarray(m[name]) for m in in_maps], axis=0)
            args.append(jax.device_put(cat, sh))
        for z in self.zero_outs:
            zz = np.zeros((self.n_cores * z.shape[0], *z.shape[1:]), z.dtype)
            args.append(jax.device_put(zz, sh))
        return args

    def run(self, args):
        return self.fn(*args)

    def results(self, out_arrs):
        res = []
        for c in range(self.n_cores):
            res.append({
                name: np.asarray(out_arrs[i]).reshape(
                    self.n_cores, *self.out_avals[i].shape)[c]
                for i, name in enumerate(self.out_names)})
        return res


# ------------------------------------------------------------- kernel()
_CACHE = {}
_RUNNERS = {}
LAST_ARGS = None
LAST_LAUNCH_NS = None


def build_in_map(cfg, cur, sched, assign, slots, lw, ohP, c):
    NBLK = int(np.sum(sched))
    S = NBLK * 128
    sids, dloc = slots[c]
    pad = sids < 0
    xs = cur[np.where(pad, 0, sids)]
    xs[pad] = 0

    # xeP [128, NBLK*66]: partition-major rows of xs
    xeP = np.zeros((NBLK, 128, 66), np.float16)
    xeP[:, :, :cfg.D] = xs.reshape(NBLK, 128, cfg.D)
    xeP[:, :, cfg.D] = (~pad).astype(np.float16).reshape(NBLK, 128)
    xeP = np.ascontiguousarray(
        xeP.transpose(1, 0, 2).reshape(128, NBLK * 66))
    # dst features: block -> per-core strip slot -> global strip
    boff = np.concatenate([[0], np.cumsum(sched)]).astype(np.int64)
    sstrip = np.searchsorted(boff[1:], np.arange(NBLK), side="right")
    gs_of_slot = np.repeat(np.clip(assign[c][sstrip], 0, None), 128)
    dlg = np.where(pad, 0, gs_of_slot * cfg.W + dloc)
    xd = cur[np.minimum(dlg, cfg.N - 1)]
    xd[pad] = 0
    # stacked layout: rows 0:64 = xs.T, 64:128 = xd.T (pads killed by onehot)
    xsd = np.empty((2 * cfg.D, S), np.float16)
    xsd[:cfg.D] = xs.T
    xsd[cfg.D:] = xd.T
    return {
        "xsd": xsd, "xeP": xeP, "ohP": ohP[c],
        "wsd": lw["wsd"], "pbias": lw["pbias"], "A": lw["A"],
        "R2_0": lw["R2_0"], "R2_1": lw["R2_1"],
        "biasF": lw["biasF"],
    }


def prep_all(cfg, src, dst):
    sched, assign, slots = _prep_edges(cfg, src, dst)
    NBLK = int(np.sum(sched))
    ohP = []
    for c in range(cfg.NCORES):
        sids, dloc = slots[c]
        dlocP = dloc.reshape(NBLK, 128).T  # [128, NBLK]
        oh = (dlocP[:, :, None] ==
              np.arange(cfg.W, dtype=np.int64)[None, None, :])
        ohP.append(np.ascontiguousarray(
            oh.reshape(128, NBLK * cfg.W).astype(NP8)))
    return sched, assign, slots, ohP


def unpack_core(cfg, a, assign_c):
    """[128, NS*C] device output (slot order) -> list of (gs, [64, C])."""
    NS = cfg.NPAIR
    dev = a.reshape(2, cfg.W, NS, cfg.C).transpose(2, 0, 1, 3)
    dev = dev.reshape(cfg.NSTRIP, cfg.W, cfg.C)
    return dev


def unpack_full(cfg, res_list, assign, key):
    out = np.empty((cfg.GSTRIP * cfg.W, cfg.C), np.float32)
    for c in range(cfg.NCORES):
        dev = unpack_core(cfg, res_list[c][key], assign[c])
        sel = assign[c] >= 0
        gs = assign[c][sel]
        out[(gs[:, None] * cfg.W + np.arange(cfg.W)[None, :]).ravel()] = \
            dev[np.flatnonzero(sel)].reshape(-1, cfg.C)
    return out[:cfg.N]


def kernel(embeded_nodes_features, edges_connectivity, Wl, bl, Wr, br, att, bias):
    global LAST_LAUNCH_NS
    cfg = CFG
    x = np.asarray(embeded_nodes_features, np.float32)
    ec = np.asarray(edges_connectivity)
    src = np.concatenate([ec[0], np.arange(cfg.N, dtype=ec.dtype)]).astype(np.int64)
    dst = np.concatenate([ec[1], np.arange(cfg.N, dtype=ec.dtype)]).astype(np.int64)
    Wl = np.asarray(Wl, np.float32)
    bl = np.asarray(bl, np.float32)
    Wr = np.asarray(Wr, np.float32)
    br = np.asarray(br, np.float32)
    att = np.asarray(att, np.float32)
    bias = np.asarray(bias, np.float32)
    L = Wl.shape[0]
    has_bias = bool(np.any(bl) or np.any(br))

    sched, assign, slots, ohP = prep_all(cfg, src, dst)
    lws = [_prep_layer_weights(cfg, Wl[i], bl[i], Wr[i], br[i], att[i], bias[i])
           for i in range(L)]

    cur = x.astype(np.float16)
    out_full = None
    _launch_ns = []
    _args_hist = []
    for i in range(L):
        fin = (i == L - 1)
        key = (tuple(int(b) for b in sched), has_bias, fin)
        if key not in _CACHE:
            _CACHE[key] = build_program(cfg, sched, has_bias, fin)
        prog = _CACHE[key]
        lw = lws[i]
        in_maps = [build_in_map(cfg, cur, sched, assign, slots, lw, ohP, c)
                   for c in range(cfg.NCORES)]
        if key not in _RUNNERS:
            _RUNNERS[key] = Runner(prog, cfg.NCORES)
        runner = _RUNNERS[key]
        args = runner.put(in_maps)
        _args_hist.append(args)
        t0 = time.time()
        outs = runner.run(args)
        import jax
        jax.block_until_ready(outs)
        _launch_ns.append(int((time.time() - t0) * 1e9))
        res = runner.results(outs)
        raw = unpack_full(cfg, res, assign, "out_raw")
        actv = unpack_full(cfg, res, assign, "out_act").astype(np.float16)
        out_full = raw
        cur = actv
    LAST_LAUNCH_NS = _launch_ns
    global LAST_ARGS
    LAST_ARGS = _args_hist
    return out_full.astype(np.float32)


# revision 20
# speedup vs baseline: 1.0196x; 1.0196x over previous
"""Trainium2 Bass kernel for a 2-layer GATv2 (nn_GAT_40372692582770).

Gather-free, PE-centric design (v2):
  - Global 64-dst strips LPT-assigned to 8 cores (balances both total load
    and the per-position sorted-count profile, so the shared SPMD block
    schedule sched[i] = max_c cnt_sorted[c][i] has ~2-4% padding).
  - Host ships, per layer, per-edge feature columns (halo exchange
    materialized host-side; the graph is static):
      xsd [128, S]       = x[src_e] / x[dst_e] columns           (fp16)
      xeP [128, NBLK*66] = x[src_e] rows + valid col (P-major)   (fp16)
      ohP [128, NBLK*64] = one-hot(dstloc) precomputed           (fp8, exact)
  - Device pipeline, 2-iteration software stages (per pair of strips):
      frontA(p): z = Wsd^T xsd (PE, 512-col groups) ->
                 L = lrelu(z) split Scalar(Prelu) / Vector(2-op relu trick)
      mid(q=p-2): e-dot blocks (lhsT=L_blk, rhs=A) INTERLEAVED with the
                 gt scatter chain of pair q-2 (hides e-dot LDWEIGHTS under
                 gt matmul execution); then w = exp(e-2) (Scalar);
                 woh = oh * w (Vector/GpSimd split)
      gt chain:  gt[j,128] += xe_blk^T @ woh_blk   (PE, per strip, PSUM)
      back2(r=p-5): sp[128,130] = sum_h gts_h^T @ R2_h (PE)
                 cols = [num_h0 | num_h1 | den_0 den_1]
  - Finalize (interleaved in batches): alpha-normalize, head-mean, +bias,
    gelu -> out_raw fp32 + out_act fp16, [128, NS*C] P-major in core-slot
    order (host unpermutes via the strip assignment).

One program serves both layers (weights are inputs); compiled once per
(block schedule, has_bias).
"""
import os
import sys
import time

sys.path.insert(0, "/opt/trn_rl_repo")

import numpy as np

import concourse.bass as bass
import concourse.mybir as mybir
import concourse.tile as tile
from concourse import bacc
from concourse.bass_utils import run_bass_kernel_spmd

class Cfg:
    N = 100000
    D = 64
    H = 2
    C = 64
    NCORES = 8
    W = 64             # dst nodes per strip
    ESHIFT = -2.0      # exp bias
    NFIN = 8           # finalize batches

    @property
    def GSTRIP(self):
        return (self.N + self.W - 1) // self.W   # global strips (1563)

    @property
    def NSTRIP(self):
        return (self.GSTRIP + self.NCORES - 1) // self.NCORES  # per core (196)

    @property
    def NPAIR(self):
        return self.NSTRIP // 2

    @property
    def HC(self):
        return self.H * self.C


CFG = Cfg()
FP16 = mybir.dt.float16
FP32 = mybir.dt.float32
FP8 = mybir.dt.float8e4
NP8 = mybir.dt.np(mybir.dt.float8e4)
AF = mybir.ActivationFunctionType
ALU = mybir.AluOpType
PM = mybir.MatmulPerfMode


# ------------------------------------------------------------- host prep
def _prep_edges(cfg, src, dst):
    """Global LPT strip->core assignment; route+sort edges; shared schedule.

    Returns (sched [NSTRIP], assign [NCORES, NSTRIP] of global strip ids
    (-1 = empty pad slot), slots: per-core (srcids [S], dloc [S]);
    pad slots src=-1 dloc=255).
    """
    GS = cfg.GSTRIP
    NSTRIP = cfg.NSTRIP
    shift = int(np.log2(cfg.W))
    strip = (dst >> shift).astype(np.int64)
    scnt = np.bincount(strip, minlength=GS).astype(np.int64)
    order = np.argsort(-scnt, kind="stable")
    loads = np.zeros(cfg.NCORES, np.int64)
    nst = np.zeros(cfg.NCORES, np.int64)
    assign = np.full((cfg.NCORES, NSTRIP), -1, np.int64)
    core_of_strip = np.zeros(GS, np.int64)
    BIG = np.int64(1) << 60
    for gs in order:
        cand = np.where(nst < NSTRIP, loads, BIG)
        c = int(np.argmin(cand))
        assign[c, nst[c]] = gs
        core_of_strip[gs] = c
        loads[c] += scnt[gs]
        nst[c] += 1
    # per-core slot counts (assign rows are in decreasing-count order)
    cnt_sorted = np.where(assign >= 0, scnt[np.clip(assign, 0, GS - 1)], 0)
    sched = np.maximum(1, (cnt_sorted.max(axis=0) + 127) // 128).astype(int)
    boff = np.concatenate([[0], np.cumsum(sched)])
    nslot = int(boff[-1]) * 128

    slot_of_strip = np.zeros(GS, np.int64)
    for c in range(cfg.NCORES):
        sel = assign[c] >= 0
        slot_of_strip[assign[c][sel]] = np.flatnonzero(sel)

    ecore = core_of_strip[strip]
    eslot = slot_of_strip[strip]
    dloc_all = (dst & (cfg.W - 1)).astype(np.int64)
    out = []
    for c in range(cfg.NCORES):
        sel = np.flatnonzero(ecore == c)
        o = np.argsort(eslot[sel], kind="stable")
        sel = sel[o]
        s = src[sel].astype(np.int64)
        d = dloc_all[sel]
        islot = eslot[sel]
        # position within each slot-run
        runstart = np.concatenate([[0], np.cumsum(
            np.bincount(islot, minlength=NSTRIP))])[:-1]
        pos = np.arange(len(sel)) - runstart[islot]
        dest = boff[islot] * 128 + pos
        sids = np.full(nslot, -1, np.int64)
        dloc = np.full(nslot, 255, np.int64)
        sids[dest] = s
        dloc[dest] = d
        out.append((sids, dloc))
    return sched, assign, out


def _prep_layer_weights(cfg, Wl, bl, Wr, br, att, bias):
    D, H, C = cfg.D, cfg.H, cfg.C
    HC = cfg.HC
    wsd = np.concatenate([Wl, Wr], axis=0)  # [128, HC]
    pbias = (bl + br).astype(np.float32)[:, None]
    A = np.zeros((HC, H), np.float64)
    for h in range(H):
        A[h * C:(h + 1) * C, h] = att[h]
    R2 = np.zeros((H, 66, 130), np.float64)
    for h in range(H):
        R2[h, :D, h * C:(h + 1) * C] = Wl[:, h * C:(h + 1) * C]
        R2[h, D, h * C:(h + 1) * C] = bl[h * C:(h + 1) * C]
        R2[h, D, HC + h] = 1.0          # denominator column
    biasF = np.tile(bias.astype(np.float32)[None, :], (128, 1))
    return {
        "wsd": wsd.astype(np.float16), "pbias": pbias,
        "A": A.astype(np.float16),
        "R2_0": R2[0].astype(np.float16), "R2_1": R2[1].astype(np.float16),
        "biasF": biasF,
    }


# --------------------------------------------------------- program build
def build_program(cfg, sched, has_bias, final_layer):
    D, H, C = cfg.D, cfg.H, cfg.C
    HC = cfg.HC
    W = cfg.W
    sched = [int(b) for b in sched]
    boff = [0]
    for b in sched:
        boff.append(boff[-1] + b)
    NBLK = boff[-1]
    S = NBLK * 128
    NS = cfg.NPAIR
    RROW = HC + 2
    BPMAX = max(sched[2 * p] + sched[2 * p + 1] for p in range(NS))
    MBA = max(sched)
    CSMAX = BPMAX * 128

    # finalize batches (front-loaded so the post-loop tail is small)
    fin_after = {}
    p0 = 0
    for frac in (16, 32, 48, 62, 74, 84, 90, 95, 98):
        pe_ = (NS * frac) // 98
        if pe_ > p0:
            fin_after[pe_ - 1] = (p0, pe_)
            p0 = pe_
    assert p0 == NS

    nc = bacc.Bacc("TRN2", target_bir_lowering=False, debug=False,
                   num_devices=cfg.NCORES)

    xsd = nc.declare_dram_parameter("xsd", [2 * D, S], FP16, isOutput=False)
    xeP = nc.declare_dram_parameter("xeP", [128, NBLK * 66], FP16, isOutput=False)
    ohD = nc.declare_dram_parameter("ohP", [128, NBLK * W], FP8, isOutput=False)
    wsd = nc.declare_dram_parameter("wsd", [2 * D, HC], FP16, isOutput=False)
    pbias = nc.declare_dram_parameter("pbias", [HC, 1], FP32, isOutput=False)
    Amat = nc.declare_dram_parameter("A", [HC, H], FP16, isOutput=False)
    R2_0 = nc.declare_dram_parameter("R2_0", [66, 130], FP16, isOutput=False)
    R2_1 = nc.declare_dram_parameter("R2_1", [66, 130], FP16, isOutput=False)
    biasF = nc.declare_dram_parameter("biasF", [128, C], FP32, isOutput=False)
    out_raw = nc.declare_dram_parameter("out_raw", [128, NS * C], FP32,
                                        isOutput=True)
    out_act = nc.declare_dram_parameter("out_act", [128, NS * C], FP16,
                                        isOutput=True)

    with tile.TileContext(nc) as tc:
        with (
            tc.tile_pool(name="const", bufs=1) as cpool,
            tc.tile_pool(name="stash", bufs=1) as stpool,
            tc.tile_pool(name="fin", bufs=1) as fpool,
        ):
            wsd_t = cpool.tile([2 * D, HC], FP16)
            nc.sync.dma_start(out=wsd_t[:], in_=wsd[:, :])
            pbias_t = cpool.tile([HC, 1], FP32)
            nc.sync.dma_start(out=pbias_t[:], in_=pbias[:, :])
            A_t = cpool.tile([HC, H], FP16)
            nc.sync.dma_start(out=A_t[:], in_=Amat[:, :])
            r2_t = [cpool.tile([66, 130], FP16, tag=f"r2{h}", name=f"r2{h}")
                    for h in range(H)]
            nc.sync.dma_start(out=r2_t[0][:], in_=R2_0[:, :])
            nc.sync.dma_start(out=r2_t[1][:], in_=R2_1[:, :])
            ebias_t = cpool.tile([128, 1], FP32)
            nc.vector.memset(ebias_t[:], cfg.ESHIFT)
            bias_t = cpool.tile([128, C], FP32)
            nc.sync.dma_start(out=bias_t[:], in_=biasF[:, :])
            # warm the ACT tables during initial DMA (Prelu/Exp/Sigmoid
            # loads otherwise stall the first pipeline iterations)
            warm_t = cpool.tile([128, 4], FP32)
            nc.vector.memset(warm_t[:, 0:1], 0.0)
            nc.scalar.activation(out=warm_t[:, 1:2], in_=warm_t[:, 0:1],
                                 func=AF.Prelu, alpha=0.2)
            nc.scalar.activation(out=warm_t[:, 2:3], in_=warm_t[:, 0:1],
                                 func=AF.Exp)
            nc.scalar.copy(warm_t[:, 3:4], warm_t[:, 0:1])
            if not final_layer:
                nc.scalar.activation(out=warm_t[:, 3:4], in_=warm_t[:, 0:1],
                                     func=AF.Sigmoid)
                nc.scalar.square(warm_t[:, 3:4], warm_t[:, 0:1])

            stash = stpool.tile([128, NS * RROW], FP32)
            sv = stash[:].rearrange("p (s w) -> p s w", w=RROW)

            NSBMAX = max(p1 - p0 for p0, p1 in fin_after.values())

            with (
                tc.tile_pool(name="eg", bufs=7) as egpool,
                tc.tile_pool(name="ez", bufs=7) as ezpool,
                tc.tile_pool(name="esm", bufs=6) as smpool,
                tc.tile_pool(name="fb", bufs=2) as fbpool,
                tc.tile_pool(name="zps", bufs=2, space="PSUM") as zpspool,
                tc.tile_pool(name="eps", bufs=2, space="PSUM") as epspool,
                tc.tile_pool(name="gps", bufs=2, space="PSUM") as gpspool,
                tc.tile_pool(name="sps", bufs=2, space="PSUM") as spspool,
            ):
                def frontA(pr):
                    bA = sched[2 * pr]
                    bB = sched[2 * pr + 1]
                    bp = bA + bB
                    b0 = boff[2 * pr]
                    c0 = b0 * 128
                    CS = bp * 128
                    xsd_t = egpool.tile([2 * D, CSMAX], FP16, tag="xsd")
                    nc.sync.dma_start(out=xsd_t[:, :CS],
                                      in_=xsd[:, c0:c0 + CS])
                    xe_t = egpool.tile([128, BPMAX * 66], FP16, tag="xe")
                    nc.sync.dma_start(out=xe_t[:, :bp * 66],
                                      in_=xeP[:, b0 * 66:(b0 + bp) * 66])
                    xev = xe_t[:, :bp * 66].rearrange("p (b w) -> p b w", w=66)
                    oh_t = egpool.tile([128, BPMAX * W], FP8, tag="oh")
                    nc.sync.dma_start(out=oh_t[:, :bp * W],
                                      in_=ohD[:, b0 * W:(b0 + bp) * W])
                    ohv = oh_t[:, :bp * W].rearrange("p (b s) -> p b s", s=W)

                    # zT feature-major in groups of <=512; L = lrelu(zT)
                    L = ezpool.tile([128, CSMAX], FP16, tag="L")
                    ngrp = (CS + 511) // 512
                    for g in range(ngrp):
                        g0 = g * 512
                        gw = min(512, CS - g0)
                        zp = zpspool.tile([128, 512], FP32, tag="zp")
                        nc.tensor.matmul(zp[:, :gw], lhsT=wsd_t[:],
                                         rhs=xsd_t[:, g0:g0 + gw],
                                         start=True, stop=True)
                        nc.scalar.activation(out=L[:, g0:g0 + gw],
                                             in_=zp[:, :gw], func=AF.Prelu,
                                             bias=pbias_t[:], alpha=0.2)
                    return (pr, bA, bB, xev, ohv, L, bp)

                def mid(stA, stW):
                    # e-dot blocks of pair q interleaved with gt chain of
                    # pair q-2; then exp(q), woh(q); gts-copy(q-2).
                    gts = None
                    ed_mms = []
                    ep = None
                    if stA is not None:
                        (pr, bA, bB, xev, ohv, L, bp) = stA
                        ep = epspool.tile([128, 2 * BPMAX], FP32, tag="ep")
                        for b in range(bp):
                            ed_mms.append((L, b))
                    gt_mms = []
                    gt_ps = {}
                    if stW is not None:
                        (prw, bAw, bBw, xevw, wohAvw, wohBvw, bVw) = stW

                        def whv(blk):
                            if blk < bVw:
                                return wohAvw[:, blk, :, :]
                            return wohBvw[:, blk - bVw, :, :]

                        gts = smpool.tile([66, 256], FP16, tag="gts")
                        for half, nb in ((0, bAw), (1, bBw)):
                            gt = gpspool.tile([66, 128], FP32, tag="gt")
                            gt_ps[half] = gt
                            base = 0 if half == 0 else bAw
                            for b in range(nb):
                                gt_mms.append((gt, base + b, b == 0,
                                               b == nb - 1))
                    # interleave on the PE queue: gt first (its matmul hides
                    # the following e-dot LDWEIGHTS)
                    ng, ne = len(gt_mms), len(ed_mms)
                    for i in range(max(ng, ne)):
                        if i < ng:
                            gt, blk, st, sp_ = gt_mms[i]
                            nc.tensor.matmul(
                                gt[:], lhsT=xevw[:, blk, :],
                                rhs=whv(blk), start=st, stop=sp_)
                        if i < ne:
                            L, b = ed_mms[i]
                            nc.tensor.matmul(ep[:, 2 * b:2 * b + 2],
                                             lhsT=L[:, b * 128:(b + 1) * 128],
                                             rhs=A_t[:], start=True, stop=True)
                    stW2 = None
                    if stA is not None:
                        (pr, bA, bB, xev, ohv, L, bp) = stA
                        w_t = smpool.tile([128, 2 * BPMAX], FP16, tag="w")
                        wv = w_t[:].rearrange("p (b k) -> p b k", k=2)
                        nc.scalar.activation(out=w_t[:, :2 * bp],
                                             in_=ep[:, :2 * bp], func=AF.Exp,
                                             bias=ebias_t[:])
                        # woh [128, b, 2, 64]  (V/G split by blocks)
                        bV = max(1, min(bp, int(round(bp * 0.55))))
                        bG = bp - bV
                        wohA = ezpool.tile([128, BPMAX * 2 * W], FP16,
                                           tag="wohA")
                        wohAv = wohA[:].rearrange("p (b h s) -> p b h s",
                                                  h=2, s=W)
                        wohB = ezpool.tile([128, BPMAX * 2 * W], FP16,
                                           tag="wohB")
                        wohBv = wohB[:].rearrange("p (b h s) -> p b h s",
                                                  h=2, s=W)
                        nc.vector.tensor_tensor(
                            out=wohAv[:, :bV, :, :],
                            in0=ohv[:, :bV, :].unsqueeze(2).to_broadcast(
                                [128, bV, 2, W]),
                            in1=wv[:, :bV, :].unsqueeze(3).to_broadcast(
                                [128, bV, 2, W]),
                            op=ALU.mult)
                        if bG > 0:
                            nc.gpsimd.tensor_tensor(
                                out=wohBv[:, :bG, :, :],
                                in0=ohv[:, bV:bp, :].unsqueeze(2).to_broadcast(
                                    [128, bG, 2, W]),
                                in1=wv[:, bV:bp, :].unsqueeze(3).to_broadcast(
                                    [128, bG, 2, W]),
                                op=ALU.mult)
                        stW2 = (pr, bA, bB, xev, wohAv, wohBv, bV)
                    # gts copy of pair q-2
                    if stW is not None:
                        gtsv = gts[:].rearrange("p (h s w) -> p h s w",
                                                h=2, s=2, w=W)
                        for half in (0, 1):
                            gtv = gt_ps[half][:].rearrange(
                                "p (h w) -> p h w", h=2, w=W)
                            dst_sl = gtsv[:, :, half, :]
                            if half == 0:
                                nc.vector.tensor_copy(dst_sl, gtv[:, :, :])
                            else:
                                nc.scalar.copy(dst_sl, gtv[:, :, :])
                        return (prw, gts), stW2
                    return None, stW2

                def back2(st2):
                    pr, gts = st2
                    sp = spspool.tile([128, RROW], FP32, tag="sp")
                    nc.tensor.matmul(sp[:], lhsT=gts[:, 0:128],
                                     rhs=r2_t[0][:], start=True, stop=False)
                    nc.tensor.matmul(sp[:], lhsT=gts[:, 128:256],
                                     rhs=r2_t[1][:], start=False, stop=True)
                    dst_sl = stash[:, pr * RROW:(pr + 1) * RROW]
                    nc.vector.tensor_copy(dst_sl, sp[:])

                def finalize(p0, p1):
                    NSb = p1 - p0
                    sl = slice(p0, p1)
                    rec = fbpool.tile([128, NSBMAX * 2], FP32, tag="rec")
                    recv = rec[:].rearrange("p (s k) -> p s k", k=2)
                    tmean = fbpool.tile([128, NSBMAX * C], FP32, tag="tm")
                    tmv = tmean[:].rearrange("p (s c) -> p s c", c=C)
                    tm = tmv[:, :NSb, :]
                    cub = fbpool.tile([128, NSBMAX * C], FP32, tag="cub")
                    cv = cub[:].rearrange("p (s c) -> p s c", c=C)[:, :NSb, :]
                    outg = fbpool.tile([128, NSBMAX * C], FP16, tag="outg")
                    ogv = outg[:].rearrange("p (s c) -> p s c", c=C)[:, :NSb, :]
                    nc.vector.reciprocal(out=recv[:, :NSb, :],
                                         in_=sv[:, sl, HC:HC + 2])
                    nc.vector.tensor_tensor(
                        out=tm[:, :, :], in0=sv[:, sl, 0:C],
                        in1=recv[:, :NSb, 0:1].to_broadcast([128, NSb, C]),
                        op=ALU.mult)
                    nc.gpsimd.tensor_tensor(
                        out=cv[:, :, :], in0=sv[:, sl, C:2 * C],
                        in1=recv[:, :NSb, 1:2].to_broadcast([128, NSb, C]),
                        op=ALU.mult)
                    nc.vector.tensor_tensor(out=tm[:, :, :],
                                            in0=tm[:, :, :],
                                            in1=cv[:, :, :], op=ALU.add)
                    # tm = 0.5*tm + bias
                    nc.vector.scalar_tensor_tensor(
                        out=tm[:, :, :], in0=tm[:, :, :], scalar=0.5,
                        in1=bias_t[:].unsqueeze(1).to_broadcast(
                            [128, NSb, C]),
                        op0=ALU.mult, op1=ALU.add)
                    if final_layer:
                        nc.scalar.dma_start(
                            out=out_raw[:, p0 * C:p1 * C],
                            in_=tmean[:, :NSb * C])
                        return
                    # gelu_tanh(x) = x*sigmoid(2*sqrt(2/pi)*(x+0.044715x^3))
                    nc.scalar.square(cv[:, :, :], tm[:, :, :])
                    nc.gpsimd.tensor_tensor(out=cv[:, :, :],
                                            in0=cv[:, :, :],
                                            in1=tm[:, :, :], op=ALU.mult)
                    nc.vector.scalar_tensor_tensor(
                        out=cv[:, :, :], in0=cv[:, :, :],
                        scalar=0.044715, in1=tm[:, :, :],
                        op0=ALU.mult, op1=ALU.add)
                    nc.scalar.activation(out=cv[:, :, :],
                                         in_=cv[:, :, :],
                                         func=AF.Sigmoid,
                                         scale=1.5957691216057308)
                    nc.vector.tensor_tensor(out=ogv[:, :, :],
                                            in0=cv[:, :, :],
                                            in1=tm[:, :, :], op=ALU.mult)
                    nc.scalar.dma_start(
                        out=out_act[:, p0 * C:p1 * C],
                        in_=outg[:, :NSb * C])

                # pipeline: frontA(i) | mid: edot(i-2)+gt(i-4) | back2(i-5)
                stA = {}
                stW = {}
                gts_q = {}
                for i in range(NS + 5):
                    if i < NS:
                        stA[i] = frontA(i)
                    q = i - 2
                    if 0 <= q < NS + 2:
                        a = stA.pop(q, None)
                        wst = stW.pop(q - 2, None)
                        g, w2 = mid(a, wst)
                        if w2 is not None:
                            stW[q] = w2
                        if g is not None:
                            gts_q[g[0]] = g
                    r = i - 4
                    if 0 <= r < NS and r in gts_q:
                        back2(gts_q.pop(r))
                        if r in fin_after:
                            finalize(*fin_after[r])
                # drain any stragglers (shouldn't happen, but be safe)
                for r in sorted(gts_q):
                    back2(gts_q.pop(r))
                    if r in fin_after:
                        finalize(*fin_after[r])

    nc.compile()
    return nc


# gt-half views: mid() passes (wohAv, wohBv, bV) — the gt chain maps
# logical block b to wohAv[b] if b < bV else wohBv[b - bV]. Half A spans
# blocks [0, bA), half B [bA, bp). Resolve in a helper at issue time.
def _woh_view(wohAv, wohBv, bV, blk):
    if blk < bV:
        return wohAv[:, blk, :, :]
    return wohBv[:, blk - bV, :, :]


# ----------------------------------------------------- persistent runner
class Runner:
    """Jit-compiled SPMD callable with reusable device inputs (no donation)."""

    def __init__(self, nc, n_cores):
        import jax
        import concourse.mybir as mb
        from concourse import bass2jax
        from jax.experimental.shard_map import shard_map
        from jax.sharding import Mesh, PartitionSpec
        bass2jax.install_neuronx_cc_hook()
        self.nc = nc
        self.n_cores = n_cores
        in_names, out_names, out_avals, zero_outs = [], [], [], []
        for alloc in nc.m.functions[0].allocations:
            if not isinstance(alloc, mb.MemoryLocationSet):
                continue
            name = alloc.memorylocations[0].name
            if alloc.kind == "ExternalInput":
                in_names.append(name)
            elif alloc.kind == "ExternalOutput":
                out_names.append(name)
                shape = tuple(alloc.tensor_shape)
                dtype = mb.dt.np(alloc.dtype)
                out_avals.append(jax.core.ShapedArray(shape, dtype))
                zero_outs.append(np.zeros(shape, dtype))
        pt = nc.partition_id_tensor
        self.pname = pt.name if pt else None
        if self.pname in in_names:
            in_names.remove(self.pname)
        self.in_names = list(in_names)
        self.out_names = list(out_names)
        self.out_avals = out_avals
        self.zero_outs = zero_outs
        all_in = list(in_names) + list(out_names)
        if self.pname:
            all_in.append(self.pname)

        def _body(*args):
            operands = list(args)
            if self.pname:
                operands.append(bass2jax.partition_id_tensor())
            outs = bass2jax._bass_exec_p.bind(
                *operands,
                out_avals=tuple(out_avals),
                in_names=tuple(all_in),
                out_names=tuple(out_names),
                lowering_input_output_aliases=(),
                sim_require_finite=True,
                sim_require_nnan=True,
                nc=nc,
            )
            return tuple(outs)

        devices = jax.devices()[:n_cores]
        self.mesh = Mesh(np.asarray(devices), ("core",))
        np_in = (PartitionSpec("core"),) * (len(in_names) + len(out_names))
        np_out = (PartitionSpec("core"),) * len(out_names)
        self.fn = jax.jit(shard_map(_body, mesh=self.mesh, in_specs=np_in,
                                    out_specs=np_out, check_rep=False),
                          keep_unused=True)

    def put(self, in_maps):
        """Concat per-core inputs and move to device. Returns arg list."""
        import jax
        from jax.sharding import NamedSharding, PartitionSpec
        sh = NamedSharding(self.mesh, PartitionSpec("core"))
        args = []
        for name in self.in_names:
            cat = np.concatenate([np.asarray(m[name]) for m in in_maps], axis=0)
            args.append(jax.device_put(cat, sh))
        for z in self.zero_outs:
            zz = np.zeros((self.n_cores * z.shape[0], *z.shape[1:]), z.dtype)
            args.append(jax.device_put(zz, sh))
        return args

    def run(self, args):
        return self.fn(*args)

    def results(self, out_arrs):
        res = []
        for c in range(self.n_cores):
            res.append({
                name: np.asarray(out_arrs[i]).reshape(
                    self.n_cores, *self.out_avals[i].shape)[c]
                for i, name in enumerate(self.out_names)})
        return res


# ------------------------------------------------------------- kernel()
_CACHE = {}
_RUNNERS = {}
LAST_ARGS = None
LAST_LAUNCH_NS = None


def build_in_map(cfg, cur, sched, assign, slots, lw, ohP, c):
    NBLK = int(np.sum(sched))
    S = NBLK * 128
    sids, dloc = slots[c]
    pad = sids < 0
    xs = cur[np.where(pad, 0, sids)]
    xs[pad] = 0

    # xeP [128, NBLK*66]: partition-major rows of xs
    xeP = np.zeros((NBLK, 128, 66), np.float16)
    xeP[:, :, :cfg.D] = xs.reshape(NBLK, 128, cfg.D)
    xeP[:, :, cfg.D] = (~pad).astype(np.float16).reshape(NBLK, 128)
    xeP = np.ascontiguousarray(
        xeP.transpose(1, 0, 2).reshape(128, NBLK * 66))
    # dst features: block -> per-core strip slot -> global strip
    boff = np.concatenate([[0], np.cumsum(sched)]).astype(np.int64)
    sstrip = np.searchsorted(boff[1:], np.arange(NBLK), side="right")
    gs_of_slot = np.repeat(np.clip(assign[c][sstrip], 0, None), 128)
    dlg = np.where(pad, 0, gs_of_slot * cfg.W + dloc)
    xd = cur[np.minimum(dlg, cfg.N - 1)]
    xd[pad] = 0
    # stacked layout: rows 0:64 = xs.T, 64:128 = xd.T (pads killed by onehot)
    xsd = np.empty((2 * cfg.D, S), np.float16)
    xsd[:cfg.D] = xs.T
    xsd[cfg.D:] = xd.T
    return {
        "xsd": xsd, "xeP": xeP, "ohP": ohP[c],
        "wsd": lw["wsd"], "pbias": lw["pbias"], "A": lw["A"],
        "R2_0": lw["R2_0"], "R2_1": lw["R2_1"],
        "biasF": lw["biasF"],
    }


def prep_all(cfg, src, dst):
    sched, assign, slots = _prep_edges(cfg, src, dst)
    NBLK = int(np.sum(sched))
    ohP = []
    for c in range(cfg.NCORES):
        sids, dloc = slots[c]
        dlocP = dloc.reshape(NBLK, 128).T  # [128, NBLK]
        oh = (dlocP[:, :, None] ==
              np.arange(cfg.W, dtype=np.int64)[None, None, :])
        ohP.append(np.ascontiguousarray(
            oh.reshape(128, NBLK * cfg.W).astype(NP8)))
    return sched, assign, slots, ohP


def unpack_core(cfg, a, assign_c):
    """[128, NS*C] device output (slot order) -> list of (gs, [64, C])."""
    NS = cfg.NPAIR
    dev = a.reshape(2, cfg.W, NS, cfg.C).transpose(2, 0, 1, 3)
    dev = dev.reshape(cfg.NSTRIP, cfg.W, cfg.C)
    return dev


def unpack_full(cfg, res_list, assign, key):
    out = np.empty((cfg.GSTRIP * cfg.W, cfg.C), np.float32)
    for c in range(cfg.NCORES):
        dev = unpack_core(cfg, res_list[c][key], assign[c])
        sel = assign[c] >= 0
        gs = assign[c][sel]
        out[(gs[:, None] * cfg.W + np.arange(cfg.W)[None, :]).ravel()] = \
            dev[np.flatnonzero(sel)].reshape(-1, cfg.C)
    return out[:cfg.N]


def kernel(embeded_nodes_features, edges_connectivity, Wl, bl, Wr, br, att, bias):
    global LAST_LAUNCH_NS
    cfg = CFG
    x = np.asarray(embeded_nodes_features, np.float32)
    ec = np.asarray(edges_connectivity)
    src = np.concatenate([ec[0], np.arange(cfg.N, dtype=ec.dtype)]).astype(np.int64)
    dst = np.concatenate([ec[1], np.arange(cfg.N, dtype=ec.dtype)]).astype(np.int64)
    Wl = np.asarray(Wl, np.float32)
    bl = np.asarray(bl, np.float32)
    Wr = np.asarray(Wr, np.float32)
    br = np.asarray(br, np.float32)
    att = np.asarray(att, np.float32)
    bias = np.asarray(bias, np.float32)
    L = Wl.shape[0]
    has_bias = bool(np.any(bl) or np.any(br))

    sched, assign, slots, ohP = prep_all(cfg, src, dst)
    lws = [_prep_layer_weights(cfg, Wl[i], bl[i], Wr[i], br[i], att[i], bias[i])
           for i in range(L)]

    cur = x.astype(np.float16)
    out_full = None
    _launch_ns = []
    _args_hist = []
    for i in range(L):
        fin = (i == L - 1)
        key = (tuple(int(b) for b in sched), has_bias, fin)
        if key not in _CACHE:
            _CACHE[key] = build_program(cfg, sched, has_bias, fin)
        prog = _CACHE[key]
        lw = lws[i]
        in_maps = [build_in_map(cfg, cur, sched, assign, slots, lw, ohP, c)
                   for c in range(cfg.NCORES)]
        if key not in _RUNNERS:
            _RUNNERS[key] = Runner(prog, cfg.NCORES)
        runner = _RUNNERS[key]
        args = runner.put(in_maps)
        _args_hist.append(args)
        t0 = time.time()
        outs = runner.run(args)
        import jax
        jax.block_until_ready(outs)
        _launch_ns.append(int((time.time() - t0) * 1e9))
        res = runner.results(outs)
        raw = unpack_full(cfg, res, assign, "out_raw")
        actv = unpack_full(cfg, res, assign, "out_act").astype(np.float16)
        out_full = raw
        cur = actv
    LAST_LAUNCH_NS = _launch_ns
    global LAST_ARGS
    LAST_ARGS = _args_hist
    return out_full.astype(np.float32)
